# revision 30
# baseline (speedup 1.0000x reference)
"""DualGAT (2-hop, 2-graph GAT + gated fuse + MLP) on 8 Trainium2 NeuronCores.

Math used per GAT layer/head (z[v,u] = s_v + t_u):
    exp(LeakyRelu(z, 0.2)) = max(exp(z), exp(0.2 z))        (exact)
    exp(z) = P_v Q_u,  exp(0.2 z) = p_v q_u                 (separable)
    branch select c[v,u] = 1{z > 0}
So with Chat = adjT * c (one scalar_tensor_tensor per u-chunk: exact {0,1})
and G = adjT:
    numT @ [Wh|1] = P∘(Chat.T @ (Q∘[Wh|1])) + p∘((G-Chat).T @ (q∘[Wh|1]))
All fp32-exact; only the c threshold sees bf16 rounding of s/t (harmless:
mis-assigned elements have |branch difference| = O(|z|) -> 0 at threshold).

Sharding: v (attention rows) split 8 ways, 384 rows/core; u (neighbors) full.
Feature tensors downstream of attention use padded 4x17 head blocks (17th
lane = softmax denominator, ~1.0 junk after normalize); weight rows there are
zero-padded so junk never contributes.
"""

import sys
import numpy as np

for _p in ("/opt/trn_rl_repo",):
    if _p not in sys.path:
        sys.path.insert(0, _p)

import ml_dtypes

N = 3072
IN_DIM = 32
HID = 64
HEADS = 4
HD = 16
NCORES = 8
VL = N // NCORES          # 384
P = 128
UC = N // P               # 24
FP = 128                  # padded feature rows: 4 heads x 32 (16 feats, den@16, zeros)
MH = HID // 2
KROWS = [IN_DIM, FP]
BLK = 32
GOFF = [0, 72]
SOFF = [64, 136]
TOFF = [68, 140]

DEBUG = False
NO_COLLECTIVE = False

_CACHE = {}


def _build():
    import concourse.bacc as bacc
    import concourse.mybir as mybir
    from concourse.tile import TileContext

    dt = mybir.dt
    op = mybir.AluOpType
    AF = mybir.ActivationFunctionType

    nc = bacc.Bacc("TRN2", target_bir_lowering=False, debug=False,
                   num_devices=NCORES)

    def dram_in(name, shape, dtype=dt.float32):
        return nc.dram_tensor(name, list(shape), dtype, kind="ExternalInput")

    xT_d = dram_in("xT", (IN_DIM, N))
    xOwn_d = dram_in("xOwnT", (IN_DIM, VL))
    adj_d = [dram_in("adjTB_i", (P, UC * VL), dt.bfloat16),
             dram_in("adjTB_c", (P, UC * VL), dt.bfloat16)]
    W_d = [[dram_in(f"W{l}{g}", (KROWS[l], HID)) for g in range(2)] for l in range(2)]
    WT_d = [[dram_in(f"WT{l}{g}", (HID, KROWS[l])) for g in range(2)] for l in range(2)]
    A_d = [[dram_in(f"A{l}{g}", (HID, 2 * HEADS)) for g in range(2)] for l in range(2)]
    qg_d = [dram_in(f"qg{l}", (FP, 2)) for l in range(2)]
    mw1_d = dram_in("mw1", (FP, MH))
    mb1_d = dram_in("mb1", (MH, 1))
    mw2_d = dram_in("mw2", (MH, 1))
    mb2_d = dram_in("mb2", (1, 1))
    out_d = nc.dram_tensor("out", [1, VL], dt.float32, kind="ExternalOutput")
    dbg = {}
    if DEBUG:
        for nm, shp in [("d_wh", (P, UC * 144)), ("d_st", (8, VL)),
                        ("d_rr", (HEADS, VL)), ("d_cm1", (FP, VL)),
                        ("d_cm3", (FP, VL)), ("d_cpg", (FP, VL)),
                        ("d_xx", (FP, VL)), ("d_hgx", (FP, VL)),
                        ("d_he0", (FP, VL)), ("d_he1", (FP, VL)),
                        ("d_hf1", (FP, VL)), ("d_h1t", (FP, N))]:
            dbg[nm] = nc.dram_tensor(nm, list(shp), dt.float32, kind="ExternalOutput")

    # inline consts
    sel8_np = np.zeros((8, HEADS * P), dtype=np.float32)
    for h in range(HEADS):
        sel8_np[h, P * h:P * (h + 1)] = 1.0
    sel8_d = nc.inline_tensor(sel8_np.astype(ml_dtypes.bfloat16), name="sel8")
    e17_np = np.zeros((HEADS, FP), dtype=np.float32)
    for h in range(HEADS):
        e17_np[h, BLK * h:BLK * (h + 1)] = 1.0
    e17_d = nc.inline_tensor(e17_np.astype(ml_dtypes.bfloat16), name="e17")
    e17f_d = nc.inline_tensor(e17_np, name="e17f")
    ones68_d = nc.inline_tensor(np.ones((1, FP), dtype=np.float32), name="ones68")

    # persistent sbuf
    def sb(name, shape, dtype=dt.float32):
        return nc.alloc_sbuf_tensor(name, list(shape), dtype).ap()

    xT = sb("s_xT", (IN_DIM, N))
    XOWN = sb("s_xOwn", (IN_DIM, VL))
    adjTB = [sb(f"s_adjTB{g}", (P, UC * VL), dt.bfloat16) for g in range(2)]
    H1T = sb("s_H1T", (FP, N))
    WH = sb("s_WH", (P, UC * 144))
    QQ = sb("s_QQ", (P, UC * 16))
    WT_u = [[sb(f"s_WTu{g}{h}", (P, UC * 2 * BLK), dt.bfloat16) for h in range(HEADS)] for g in range(2)]
    GW = [sb(f"s_GW{g}", (P, UC * HEADS * BLK), dt.bfloat16) for g in range(2)]
    WST = sb("s_WST", (FP, 144))
    WSTB = sb("s_WSTB", (FP, 144))
    ST = [sb(f"s_ST{g}", (8, VL), dt.bfloat16) for g in range(2)]
    RR = [sb(f"s_RR{g}", (HEADS, VL)) for g in range(2)]
    CM1 = [sb(f"s_CM1_{g}", (FP, VL)) for g in range(2)]
    CM3 = [sb(f"s_CM3_{g}", (FP, VL)) for g in range(2)]
    CPG = [sb(f"s_CPG_{g}", (FP, VL)) for g in range(2)]
    HE = [sb(f"s_HE{g}", (FP, VL)) for g in range(2)]
    HF1 = sb("s_HF1", (FP, VL))
    HF2 = sb("s_HF2", (FP, VL))
    SEL8 = sb("s_sel8", (8, HEADS * P), dt.bfloat16)
    E17 = sb("s_e17", (HEADS, FP), dt.bfloat16)
    E17F = sb("s_e17f", (HEADS, FP))
    ONES68 = sb("s_ones68", (1, FP))
    QG = [sb(f"s_qg{l}", (FP, 2)) for l in range(2)]
    MW1 = sb("s_mw1", (FP, MH))
    MB1 = sb("s_mb1", (MH, 1))
    MW2 = sb("s_mw2", (MH, 1))
    MB2 = sb("s_mb2", (1, 1))
    WTSB = [[sb(f"s_WT{l}{g}", (HID, KROWS[l])) for g in range(2)] for l in range(2)]
    ASB = [[sb(f"s_A{l}{g}", (HID, 2 * HEADS)) for g in range(2)] for l in range(2)]
    WASB = [[sb(f"s_WA{l}{g}", (KROWS[l], 2 * HEADS)) for g in range(2)] for l in range(2)]

    WH_v = WH.rearrange("p (k c) -> p k c", c=144)
    QQ_v = QQ.rearrange("p (k g j h) -> p k g j h", g=2, j=2, h=HEADS)
    adj_v = [a.rearrange("p (k v) -> p k v", v=VL) for a in adjTB]
    GW_v = [g.rearrange("p (k h c) -> p k h c", h=HEADS, c=BLK) for g in GW]
    WTu_v = [[WT_u[g][h].rearrange("p (k j c) -> p k j c", j=2, c=BLK)
              for h in range(HEADS)] for g in range(2)]

    with TileContext(nc) as tc:
        with tc.tile_pool(name="work", bufs=6) as wp, \
             tc.tile_pool(name="chat", bufs=6) as chp, \
             tc.tile_pool(name="nsb", bufs=5) as nsp, \
             tc.tile_pool(name="small", bufs=6) as smp, \
             tc.tile_pool(name="ps_st", bufs=2, space="PSUM") as ps_st, \
             tc.tile_pool(name="ps_c", bufs=4, space="PSUM") as ps_c, \
             tc.tile_pool(name="ps_m", bufs=2, space="PSUM") as ps_m, \
             tc.tile_pool(name="dram", bufs=1, space="DRAM") as drp:

            # ---------- loads (small/critical first; big adjacency last) ----------
            nc.sync.dma_start(out=xT[:], in_=xT_d.ap())
            nc.sync.dma_start(out=XOWN[:], in_=xOwn_d.ap())
            nc.sync.dma_start(out=SEL8[:], in_=sel8_d.ap())
            nc.sync.dma_start(out=E17[:], in_=e17_d.ap())
            nc.sync.dma_start(out=E17F[:], in_=e17f_d.ap())
            nc.sync.dma_start(out=ONES68[:], in_=ones68_d.ap())
            for l in range(2):
                nc.sync.dma_start(out=QG[l][:], in_=qg_d[l].ap())
                for g in range(2):
                    nc.sync.dma_start(out=WTSB[l][g][:], in_=WT_d[l][g].ap())
                    nc.sync.dma_start(out=ASB[l][g][:], in_=A_d[l][g].ap())
            nc.sync.dma_start(out=MW1[:], in_=mw1_d.ap())
            nc.sync.dma_start(out=MB1[:], in_=mb1_d.ap())
            nc.sync.dma_start(out=MW2[:], in_=mw2_d.ap())
            nc.sync.dma_start(out=MB2[:], in_=mb2_d.ap())
            for g in range(2):
                nc.gpsimd.memset(GW[g][:], 0.0)
                for h in range(HEADS):
                    nc.vector.memset(WT_u[g][h][:], 0.0)

            def prep_weights(l):
                krows = KROWS[l]
                wst = WST if l == 0 else WSTB
                for g in range(2):
                    wa_ps = ps_m.tile([KROWS[1], 2 * HEADS], dt.float32,
                                      tag="m")
                    nc.tensor.matmul(wa_ps[:krows, :], WTSB[l][g][:],
                                     ASB[l][g][:], start=True, stop=True)
                    nc.sync.dma_start(out=wst[0:krows, GOFF[g]:GOFF[g] + HID],
                                      in_=W_d[l][g].ap())
                    nc.scalar.copy(wst[0:krows, SOFF[g]:SOFF[g] + 8],
                                   wa_ps[:krows, :])
                    nc.scalar.copy(WASB[l][g][:], wa_ps[:krows, :])

            def layer(l, HT, hown, hf_out):
                """One hop. HT: (krows, N) node-major features (transposed);
                hown: (krows, VL) own-slice features; hf_out: fused output."""
                krows = KROWS[l]

                wst = WST if l == 0 else WSTB

                # st+Wh per u-chunk: (krows x 128).T @ (krows x 144)
                for k in range(UC):
                    stwh = ps_st.tile([P, 144], dt.float32, tag="stwh")
                    nc.tensor.matmul(stwh[:], HT[:, P * k:P * (k + 1)],
                                     wst[0:krows, :], start=True, stop=True)
                    if k % 2 == 0:
                        nc.scalar.copy(WH_v[:, k, :], stwh[:])
                    else:
                        nc.vector.tensor_copy(out=WH_v[:, k, :], in_=stwh[:])

                if DEBUG and l == 0:
                    nc.sync.dma_start(out=dbg["d_wh"].ap(), in_=WH[:])

                # Q/q
                for g in range(2):
                    tcols = WH_v[:, :, TOFF[g]:TOFF[g] + 4]
                    nc.scalar.activation(QQ_v[:, :, g, 0, :], tcols, AF.Exp)
                    nc.scalar.activation(QQ_v[:, :, g, 1, :], tcols, AF.Exp,
                                         scale=0.2)

                # own-slice s/t rows: ST = WA.T @ hown  (8 x VL)
                for g in range(2):
                    st_ps = ps_m.tile([8, VL], dt.float32, tag="m")
                    nc.tensor.matmul(st_ps[:], WASB[l][g][:], hown[:],
                                     start=True, stop=True)
                    nc.scalar.copy(ST[g][:], st_ps[:])
                    nc.scalar.activation(RR[g][:], ST[g][0:HEADS, :], AF.Exp,
                                         scale=0.8)
                    if DEBUG and l == 0 and g == 0:
                        nc.gpsimd.dma_start(out=dbg["d_st"].ap(), in_=ST[0][:])
                        nc.gpsimd.dma_start(out=dbg["d_rr"].ap(), in_=RR[0][:])

                # weight builds
                for g in range(2):
                    nc.gpsimd.tensor_tensor(
                        out=GW_v[g][:, :, :, 0:16],
                        in0=WH_v[:, :, GOFF[g]:GOFF[g] + HID].rearrange(
                            "p k (h d) -> p k h d", d=HD),
                        in1=QQ_v[:, :, g, 1, :][:, :, :, None].to_broadcast(
                            (P, UC, HEADS, HD)),
                        op=op.mult)
                    nc.gpsimd.tensor_copy(out=GW_v[g][:, :, :, 16],
                                          in_=QQ_v[:, :, g, 1, :])
                    for h in range(HEADS):
                        nc.gpsimd.tensor_tensor(
                            out=WTu_v[g][h][:, :, :, 0:16],
                            in0=WH_v[:, :, GOFF[g] + HD * h:
                                     GOFF[g] + HD * h + HD][:, :, None, :]
                                .to_broadcast((P, UC, 2, HD)),
                            in1=QQ_v[:, :, g, :, h][:, :, :, None].to_broadcast(
                                (P, UC, 2, HD)),
                            op=op.mult)
                        nc.gpsimd.tensor_copy(out=WTu_v[g][h][:, :, :, 16],
                                              in_=QQ_v[:, :, g, :, h])

                if l == 0:
                    for g in range(2):
                        nc.gpsimd.dma_start(out=adjTB[g][:], in_=adj_d[g].ap())

                # attention units: c = 1{s+t>0} via 4x TS, mask via one
                # head-batched 2x TT per chunk, then 4 matmuls.
                for g in range(2):
                    sbs = []
                    for h in range(HEADS):
                        sb_ps = ps_st.tile([P, VL], dt.float32, tag="stwh")
                        nc.tensor.matmul(sb_ps[:],
                                         SEL8[:, P * h:P * (h + 1)],
                                         ST[g][:], start=True, stop=True)
                        s_b = nsp.tile([P, VL], dt.bfloat16, tag="ns_b")
                        nc.scalar.copy(s_b[:], sb_ps[:])
                        sbs.append(s_b)

                    psum_cs = []
                    for h in range(HEADS):
                        psum_c = ps_c.tile([2 * BLK, VL], dt.float32,
                                           tag="psum_c")
                        psum_cs.append(psum_c)
                    for k in range(UC):
                        veng = nc.gpsimd if k in (8, 16) else nc.vector
                        cb4 = chp.tile([P, HEADS, VL], dt.bfloat16, tag="cb4")
                        for h in range(HEADS):
                            nc.vector.tensor_scalar(
                                cb4[:, h, :], sbs[h][:],
                                WH_v[:, k, TOFF[g] + h:TOFF[g] + h + 1], 0.0,
                                op.add, op.is_gt)
                        chat4 = chp.tile([P, HEADS, VL], dt.bfloat16, tag="chat4")
                        veng.tensor_tensor(
                            out=chat4[:], in0=cb4[:],
                            in1=adj_v[g][:, k, :][:, None, :].to_broadcast(
                                (P, HEADS, VL)),
                            op=op.mult)
                        for h in range(HEADS):
                            nc.tensor.matmul(psum_cs[h][:],
                                             WTu_v[g][h][:, k, :, :],
                                             chat4[:, h, :], start=(k == 0),
                                             stop=(k == UC - 1))
                    for h in range(HEADS):
                        nc.scalar.copy(CM1[g][BLK * h:BLK * (h + 1), :],
                                       psum_cs[h][0:BLK, :])
                        nc.scalar.copy(CM3[g][BLK * h:BLK * (h + 1), :],
                                       psum_cs[h][BLK:2 * BLK, :])

                    # G-stream: rhs is the resident {0,1} bf16 adjacency
                    psum_g = ps_c.tile([FP, VL], dt.float32, tag="psum_c")
                    for k in range(UC):
                        nc.tensor.matmul(psum_g[:], GW_v[g][:, k, :, :],
                                         adj_v[g][:, k, :], start=(k == 0),
                                         stop=(k == UC - 1))

                    # epilogue (batched over the 4 heads)
                    nc.scalar.copy(CPG[g][:], psum_g[:])
                    if DEBUG and l == 0 and g == 0:
                        nc.sync.dma_start(out=dbg["d_cm1"].ap(), in_=CM1[0][:])
                        nc.sync.dma_start(out=dbg["d_cm3"].ap(), in_=CM3[0][:])
                        nc.sync.dma_start(out=dbg["d_cpg"].ap(), in_=CPG[0][:])
                    t4 = wp.tile([FP, VL], dt.float32, tag="w")
                    nc.vector.tensor_tensor(out=t4[:], in0=CPG[g][:],
                                            in1=CM3[g][:], op=op.subtract)
                    rb_ps = ps_m.tile([FP, VL], dt.float32, tag="m")
                    nc.tensor.matmul(rb_ps[:], E17F[:], RR[g][:],
                                     start=True, stop=True)
                    m1r = wp.tile([FP, VL], dt.float32, tag="w")
                    nc.vector.tensor_tensor(out=m1r[:], in0=CM1[g][:],
                                            in1=rb_ps[:], op=op.mult)
                    xx = wp.tile([FP, VL], dt.float32, tag="w")
                    nc.vector.tensor_tensor(out=xx[:], in0=t4[:], in1=m1r[:],
                                            op=op.add)
                    den4 = smp.tile([HEADS, VL], dt.float32, tag="s")
                    nc.sync.dma_start(out=den4[:], in_=xx[16::BLK, :])
                    rda = smp.tile([HEADS, VL], dt.float32, tag="s")
                    nc.vector.reciprocal(rda[:], den4[:])
                    rd_ps = ps_m.tile([FP, VL], dt.float32, tag="m")
                    nc.tensor.matmul(rd_ps[:], E17F[:], rda[:],
                                     start=True, stop=True)
                    hgx = wp.tile([FP, VL], dt.float32, tag="w")
                    nc.vector.tensor_tensor(out=hgx[:], in0=xx[:], in1=rd_ps[:],
                                            op=op.mult)
                    if DEBUG and l == 0 and g == 0:
                        nc.sync.dma_start(out=dbg["d_xx"].ap(), in_=xx[:])
                        nc.sync.dma_start(out=dbg["d_hgx"].ap(), in_=hgx[:])

                    # elu
                    r0 = wp.tile([FP, VL], dt.float32, tag="w")
                    nc.scalar.activation(r0[:], hgx[:], AF.Relu)
                    rn = wp.tile([FP, VL], dt.float32, tag="w")
                    nc.scalar.activation(rn[:], hgx[:], AF.Relu, scale=-1.0)
                    em = wp.tile([FP, VL], dt.float32, tag="w")
                    nc.scalar.activation(em[:], rn[:], AF.Exp, scale=-1.0)
                    nc.vector.scalar_tensor_tensor(
                        out=HE[g][:], in0=r0[:], scalar=-1.0, in1=em[:],
                        op0=op.add, op1=op.add)

                if DEBUG and l == 0:
                    nc.sync.dma_start(out=dbg["d_he0"].ap(), in_=HE[0][:])
                    nc.sync.dma_start(out=dbg["d_he1"].ap(), in_=HE[1][:])

                # fuse
                ei = []
                for g in range(2):
                    ai_ps = ps_m.tile([1, VL], dt.float32, tag="m")
                    nc.tensor.matmul(ai_ps[:], QG[l][:, g:g + 1], HE[g][:],
                                     start=True, stop=True)
                    e = smp.tile([1, VL], dt.float32, tag="s")
                    nc.scalar.activation(e[:], ai_ps[:], AF.Exp)
                    ei.append(e)
                dsum = smp.tile([1, VL], dt.float32, tag="s")
                nc.vector.tensor_tensor(out=dsum[:], in0=ei[0][:], in1=ei[1][:],
                                        op=op.add)
                rdf = smp.tile([1, VL], dt.float32, tag="s")
                nc.vector.reciprocal(rdf[:], dsum[:])
                b0 = smp.tile([1, VL], dt.float32, tag="s")
                nc.vector.tensor_tensor(out=b0[:], in0=ei[0][:], in1=rdf[:],
                                        op=op.mult)
                bib_ps = ps_m.tile([FP, VL], dt.float32, tag="m")
                nc.tensor.matmul(bib_ps[:], ONES68[:], b0[:],
                                 start=True, stop=True)
                dd = wp.tile([FP, VL], dt.float32, tag="w")
                nc.vector.tensor_tensor(out=dd[:], in0=HE[0][:], in1=HE[1][:],
                                        op=op.subtract)
                bd = wp.tile([FP, VL], dt.float32, tag="w")
                nc.vector.tensor_tensor(out=bd[:], in0=dd[:], in1=bib_ps[:],
                                        op=op.mult)
                nc.vector.tensor_tensor(out=hf_out[:], in0=HE[1][:], in1=bd[:],
                                        op=op.add)

            # ---------------- hop 1 ----------------
            prep_weights(0)
            prep_weights(1)
            layer(0, xT, XOWN, HF1)

            # all-gather H1 (feature-major)
            ag_in = drp.tile([FP, VL], dt.float32)
            ag_out = drp.tile([NCORES, FP, VL], dt.float32)
            nc.gpsimd.dma_start(out=ag_in[:], in_=HF1[:])
            if NO_COLLECTIVE:
                for c in range(NCORES):
                    nc.gpsimd.dma_start(
                        out=ag_out.opt().rearrange("c (f v) -> c f v", v=VL)[c],
                        in_=ag_in[:])
            else:
                nc.gpsimd.collective_compute(
                    "AllGather", op.bypass,
                    replica_groups=[list(range(NCORES))],
                    ins=[ag_in.opt()], outs=[ag_out.opt()])
            agv = ag_out.opt().rearrange("c (f v) -> c f v", v=VL)
            h1v = H1T.rearrange("f (c v) -> f c v", v=VL)
            for c in range(NCORES):
                nc.sync.dma_start(out=h1v[:, c, :], in_=agv[c])

            if DEBUG:
                nc.gpsimd.dma_start(out=dbg["d_hf1"].ap(), in_=HF1[:])
                nc.gpsimd.dma_start(out=dbg["d_h1t"].ap(), in_=H1T[:])

            # ---------------- hop 2 ----------------
            layer(1, H1T, HF1, HF2)

            # ---------------- MLP head ----------------
            h_ps = ps_m.tile([MH, VL], dt.float32, tag="m")
            nc.tensor.matmul(h_ps[:], MW1[:], HF2[:], start=True, stop=True)
            hd = smp.tile([MH, VL], dt.float32, tag="s")
            nc.scalar.activation(hd[:], h_ps[:], AF.Relu, bias=MB1[:])
            o_ps = ps_m.tile([1, VL], dt.float32, tag="m")
            nc.tensor.matmul(o_ps[:], MW2[:], hd[:], start=True, stop=True)
            osb = smp.tile([1, VL], dt.float32, tag="s")
            nc.scalar.activation(osb[:], o_ps[:], AF.Identity, bias=MB2[:])
            nc.sync.dma_start(out=out_d.ap(), in_=osb[:])

    nc.compile()
    return nc


def _pad_rows(w):
    out = np.zeros((FP,) + w.shape[1:], dtype=np.float32)
    for h in range(HEADS):
        out[BLK * h:BLK * h + 16] = w[16 * h:16 * h + 16]
    return out


def _ahat(a):
    A = np.zeros((HID, 2 * HEADS), dtype=np.float32)
    for h in range(HEADS):
        A[16 * h:16 * h + 16, h] = a[h, :HD]
        A[16 * h:16 * h + 16, HEADS + h] = a[h, HD:]
    return A


def _prep_adj(adj, c):
    """(N,N) int -> per-core (P, UC*VL) bf16 {0,1} chunk layout of adjT."""
    sl = adj[c * VL:(c + 1) * VL, :].T.astype(np.float32)       # (N, VL)
    sl = sl.reshape(UC, P, VL).transpose(1, 0, 2).reshape(P, UC * VL)
    return np.ascontiguousarray(sl).astype(ml_dtypes.bfloat16)


def kernel(**inputs):
    from concourse.bass_utils import run_bass_kernel_spmd

    if "nc" not in _CACHE:
        _CACHE["nc"] = _build()
    nc = _CACHE["nc"]

    f32 = np.float32
    x = np.asarray(inputs["x"], f32)
    adj = [np.asarray(inputs["adj_ind"]), np.asarray(inputs["adj_cor"])]
    W1 = [np.asarray(inputs["W1i"], f32), np.asarray(inputs["W1c"], f32)]
    W2 = [np.asarray(inputs["W2i"], f32), np.asarray(inputs["W2c"], f32)]
    A1 = [np.asarray(inputs["a1i"], f32), np.asarray(inputs["a1c"], f32)]
    A2 = [np.asarray(inputs["a2i"], f32), np.asarray(inputs["a2c"], f32)]
    q1 = [np.asarray(inputs["q1i"], f32), np.asarray(inputs["q1c"], f32)]
    q2 = [np.asarray(inputs["q2i"], f32), np.asarray(inputs["q2c"], f32)]

    common = {"xT": np.ascontiguousarray(x.T)}
    for l, (Ws, As) in enumerate(((W1, A1), (W2, A2))):
        for g in range(2):
            W = Ws[g] if l == 0 else _pad_rows(Ws[g])
            common[f"W{l}{g}"] = np.ascontiguousarray(W)
            common[f"WT{l}{g}"] = np.ascontiguousarray(W.T)
            common[f"A{l}{g}"] = _ahat(As[g])
    for l, qs in enumerate((q1, q2)):
        common[f"qg{l}"] = np.ascontiguousarray(
            np.stack([_pad_rows(qs[0][:, None])[:, 0],
                      _pad_rows(qs[1][:, None])[:, 0]], axis=1))
    common["mw1"] = _pad_rows(np.asarray(inputs["mlp_w1"], f32))
    common["mb1"] = np.ascontiguousarray(np.asarray(inputs["mlp_b1"], f32)[:, None])
    common["mw2"] = np.ascontiguousarray(np.asarray(inputs["mlp_w2"], f32))
    common["mb2"] = np.asarray(inputs["mlp_b2"], f32).reshape(1, 1)

    in_maps = []
    for c in range(NCORES):
        m = dict(common)
        m["xOwnT"] = np.ascontiguousarray(x[c * VL:(c + 1) * VL, :].T)
        m["adjTB_i"] = _prep_adj(adj[0], c)
        m["adjTB_c"] = _prep_adj(adj[1], c)
        in_maps.append(m)

    res = run_bass_kernel_spmd(nc, in_maps, core_ids=list(range(NCORES)))
    out = np.concatenate([r["out"][0] for r in res.results])[:, None]
    return out.astype(np.float32)


if __name__ == "__main__":
    _CACHE["nc"] = _build()
    print("build ok")


# revision 31
# speedup vs baseline: 1.0248x; 1.0248x over previous
"""DualGAT (2-hop, 2-graph GAT + gated fuse + MLP) on 8 Trainium2 NeuronCores.

Math used per GAT layer/head (z[v,u] = s_v + t_u):
    exp(LeakyRelu(z, 0.2)) = max(exp(z), exp(0.2 z))        (exact)
    exp(z) = P_v Q_u,  exp(0.2 z) = p_v q_u                 (separable)
    branch select c[v,u] = 1{z > 0}
So with Chat = adjT * c (one scalar_tensor_tensor per u-chunk: exact {0,1})
and G = adjT:
    numT @ [Wh|1] = P∘(Chat.T @ (Q∘[Wh|1])) + p∘((G-Chat).T @ (q∘[Wh|1]))
All fp32-exact; only the c threshold sees bf16 rounding of s/t (harmless:
mis-assigned elements have |branch difference| = O(|z|) -> 0 at threshold).

Sharding: v (attention rows) split 8 ways, 384 rows/core; u (neighbors) full.
Feature tensors downstream of attention use padded 4x17 head blocks (17th
lane = softmax denominator, ~1.0 junk after normalize); weight rows there are
zero-padded so junk never contributes.
"""

import sys
import numpy as np

for _p in ("/opt/trn_rl_repo",):
    if _p not in sys.path:
        sys.path.insert(0, _p)

import ml_dtypes

N = 3072
IN_DIM = 32
HID = 64
HEADS = 4
HD = 16
NCORES = 8
VL = N // NCORES          # 384
P = 128
UC = N // P               # 24
FP = 128                  # padded feature rows: 4 heads x 32 (16 feats, den@16, zeros)
MH = HID // 2
KROWS = [IN_DIM, FP]
BLK = 32
GOFF = [0, 72]
SOFF = [64, 136]
TOFF = [68, 140]

DEBUG = False
NO_COLLECTIVE = False

_CACHE = {}


def _build():
    import concourse.bacc as bacc
    import concourse.mybir as mybir
    from concourse.tile import TileContext

    dt = mybir.dt
    op = mybir.AluOpType
    AF = mybir.ActivationFunctionType

    nc = bacc.Bacc("TRN2", target_bir_lowering=False, debug=False,
                   num_devices=NCORES)

    def dram_in(name, shape, dtype=dt.float32):
        return nc.dram_tensor(name, list(shape), dtype, kind="ExternalInput")

    xT_d = dram_in("xT", (IN_DIM, N))
    xOwn_d = dram_in("xOwnT", (IN_DIM, VL))
    adj_d = [dram_in("adjTB_i", (P, UC * VL), dt.bfloat16),
             dram_in("adjTB_c", (P, UC * VL), dt.bfloat16)]
    W_d = [[dram_in(f"W{l}{g}", (KROWS[l], HID)) for g in range(2)] for l in range(2)]
    WT_d = [[dram_in(f"WT{l}{g}", (HID, KROWS[l])) for g in range(2)] for l in range(2)]
    A_d = [[dram_in(f"A{l}{g}", (HID, 2 * HEADS)) for g in range(2)] for l in range(2)]
    qg_d = [dram_in(f"qg{l}", (FP, 2)) for l in range(2)]
    mw1_d = dram_in("mw1", (FP, MH))
    mb1_d = dram_in("mb1", (MH, 1))
    mw2_d = dram_in("mw2", (MH, 1))
    mb2_d = dram_in("mb2", (1, 1))
    out_d = nc.dram_tensor("out", [1, VL], dt.float32, kind="ExternalOutput")
    dbg = {}
    if DEBUG:
        for nm, shp in [("d_wh", (P, UC * 144)), ("d_st", (8, VL)),
                        ("d_rr", (HEADS, VL)), ("d_cm1", (FP, VL)),
                        ("d_cm3", (FP, VL)), ("d_cpg", (FP, VL)),
                        ("d_xx", (FP, VL)), ("d_hgx", (FP, VL)),
                        ("d_he0", (FP, VL)), ("d_he1", (FP, VL)),
                        ("d_hf1", (FP, VL)), ("d_h1t", (FP, N))]:
            dbg[nm] = nc.dram_tensor(nm, list(shp), dt.float32, kind="ExternalOutput")

    # inline consts
    sel8_np = np.zeros((8, HEADS * P), dtype=np.float32)
    for h in range(HEADS):
        sel8_np[h, P * h:P * (h + 1)] = 1.0
    sel8_d = nc.inline_tensor(sel8_np.astype(ml_dtypes.bfloat16), name="sel8")
    e17_np = np.zeros((HEADS, FP), dtype=np.float32)
    for h in range(HEADS):
        e17_np[h, BLK * h:BLK * (h + 1)] = 1.0
    e17_d = nc.inline_tensor(e17_np.astype(ml_dtypes.bfloat16), name="e17")
    e17f_d = nc.inline_tensor(e17_np, name="e17f")
    ones68_d = nc.inline_tensor(np.ones((1, FP), dtype=np.float32), name="ones68")

    # persistent sbuf
    def sb(name, shape, dtype=dt.float32):
        return nc.alloc_sbuf_tensor(name, list(shape), dtype).ap()

    xT = sb("s_xT", (IN_DIM, N))
    XOWN = sb("s_xOwn", (IN_DIM, VL))
    adjTB = [sb(f"s_adjTB{g}", (P, UC * VL), dt.bfloat16) for g in range(2)]
    H1T = sb("s_H1T", (FP, N))
    WH = sb("s_WH", (P, UC * 144))
    QQ = sb("s_QQ", (P, UC * 16))
    WT_u = [[sb(f"s_WTu{g}{h}", (P, UC * 2 * BLK), dt.bfloat16) for h in range(HEADS)] for g in range(2)]
    GW = [sb(f"s_GW{g}", (P, UC * HEADS * BLK), dt.bfloat16) for g in range(2)]
    WST = sb("s_WST", (FP, 144))
    WSTB = sb("s_WSTB", (FP, 144))
    ST = [sb(f"s_ST{g}", (8, VL), dt.bfloat16) for g in range(2)]
    RR = [sb(f"s_RR{g}", (HEADS, VL)) for g in range(2)]
    CM1 = [sb(f"s_CM1_{g}", (FP, VL)) for g in range(2)]
    CM3 = [sb(f"s_CM3_{g}", (FP, VL)) for g in range(2)]
    CPG = [sb(f"s_CPG_{g}", (FP, VL)) for g in range(2)]
    HE = [sb(f"s_HE{g}", (FP, VL)) for g in range(2)]
    HF1 = sb("s_HF1", (FP, VL))
    HF2 = sb("s_HF2", (FP, VL))
    SEL8 = sb("s_sel8", (8, HEADS * P), dt.bfloat16)
    E17 = sb("s_e17", (HEADS, FP), dt.bfloat16)
    E17F = sb("s_e17f", (HEADS, FP))
    ONES68 = sb("s_ones68", (1, FP))
    QG = [sb(f"s_qg{l}", (FP, 2)) for l in range(2)]
    MW1 = sb("s_mw1", (FP, MH))
    MB1 = sb("s_mb1", (MH, 1))
    MW2 = sb("s_mw2", (MH, 1))
    MB2 = sb("s_mb2", (1, 1))
    WTSB = [[sb(f"s_WT{l}{g}", (HID, KROWS[l])) for g in range(2)] for l in range(2)]
    ASB = [[sb(f"s_A{l}{g}", (HID, 2 * HEADS)) for g in range(2)] for l in range(2)]
    WASB = [[sb(f"s_WA{l}{g}", (KROWS[l], 2 * HEADS)) for g in range(2)] for l in range(2)]

    WH_v = WH.rearrange("p (k c) -> p k c", c=144)
    QQ_v = QQ.rearrange("p (k g j h) -> p k g j h", g=2, j=2, h=HEADS)
    adj_v = [a.rearrange("p (k v) -> p k v", v=VL) for a in adjTB]
    GW_v = [g.rearrange("p (k h c) -> p k h c", h=HEADS, c=BLK) for g in GW]
    WTu_v = [[WT_u[g][h].rearrange("p (k j c) -> p k j c", j=2, c=BLK)
              for h in range(HEADS)] for g in range(2)]

    with TileContext(nc) as tc:
        with tc.tile_pool(name="work", bufs=6) as wp, \
             tc.tile_pool(name="chat", bufs=6) as chp, \
             tc.tile_pool(name="nsb", bufs=5) as nsp, \
             tc.tile_pool(name="small", bufs=6) as smp, \
             tc.tile_pool(name="ps_st", bufs=2, space="PSUM") as ps_st, \
             tc.tile_pool(name="ps_c", bufs=4, space="PSUM") as ps_c, \
             tc.tile_pool(name="ps_m", bufs=2, space="PSUM") as ps_m, \
             tc.tile_pool(name="dram", bufs=1, space="DRAM") as drp:

            # ---------- loads (small/critical first; big adjacency last) ----------
            nc.sync.dma_start(out=xT[:], in_=xT_d.ap())
            nc.sync.dma_start(out=XOWN[:], in_=xOwn_d.ap())
            nc.sync.dma_start(out=SEL8[:], in_=sel8_d.ap())
            nc.sync.dma_start(out=E17[:], in_=e17_d.ap())
            nc.sync.dma_start(out=E17F[:], in_=e17f_d.ap())
            nc.sync.dma_start(out=ONES68[:], in_=ones68_d.ap())
            for l in range(2):
                nc.sync.dma_start(out=QG[l][:], in_=qg_d[l].ap())
                for g in range(2):
                    nc.sync.dma_start(out=WTSB[l][g][:], in_=WT_d[l][g].ap())
                    nc.sync.dma_start(out=ASB[l][g][:], in_=A_d[l][g].ap())
            nc.sync.dma_start(out=MW1[:], in_=mw1_d.ap())
            nc.sync.dma_start(out=MB1[:], in_=mb1_d.ap())
            nc.sync.dma_start(out=MW2[:], in_=mw2_d.ap())
            nc.sync.dma_start(out=MB2[:], in_=mb2_d.ap())
            for g in range(2):
                nc.gpsimd.memset(GW[g][:], 0.0)
                for h in range(HEADS):
                    nc.vector.memset(WT_u[g][h][:], 0.0)

            def prep_weights(l):
                krows = KROWS[l]
                wst = WST if l == 0 else WSTB
                for g in range(2):
                    wa_ps = ps_m.tile([KROWS[1], 2 * HEADS], dt.float32,
                                      tag="m")
                    nc.tensor.matmul(wa_ps[:krows, :], WTSB[l][g][:],
                                     ASB[l][g][:], start=True, stop=True)
                    nc.sync.dma_start(out=wst[0:krows, GOFF[g]:GOFF[g] + HID],
                                      in_=W_d[l][g].ap())
                    nc.scalar.copy(wst[0:krows, SOFF[g]:SOFF[g] + 8],
                                   wa_ps[:krows, :])
                    nc.scalar.copy(WASB[l][g][:], wa_ps[:krows, :])

            def layer(l, HT, hown, hf_out):
                """One hop. HT: (krows, N) node-major features (transposed);
                hown: (krows, VL) own-slice features; hf_out: fused output."""
                krows = KROWS[l]

                wst = WST if l == 0 else WSTB

                # st+Wh per u-chunk: (krows x 128).T @ (krows x 144)
                for k in range(UC):
                    stwh = ps_st.tile([P, 144], dt.float32, tag="stwh")
                    nc.tensor.matmul(stwh[:], HT[:, P * k:P * (k + 1)],
                                     wst[0:krows, :], start=True, stop=True)
                    if k % 2 == 0:
                        nc.scalar.copy(WH_v[:, k, :], stwh[:])
                    else:
                        nc.vector.tensor_copy(out=WH_v[:, k, :], in_=stwh[:])

                if DEBUG and l == 0:
                    nc.sync.dma_start(out=dbg["d_wh"].ap(), in_=WH[:])

                # Q/q
                for g in range(2):
                    tcols = WH_v[:, :, TOFF[g]:TOFF[g] + 4]
                    nc.scalar.activation(QQ_v[:, :, g, 0, :], tcols, AF.Exp)
                    nc.scalar.activation(QQ_v[:, :, g, 1, :], tcols, AF.Exp,
                                         scale=0.2)

                # own-slice s/t rows: ST = WA.T @ hown  (8 x VL)
                for g in range(2):
                    st_ps = ps_m.tile([8, VL], dt.float32, tag="m")
                    nc.tensor.matmul(st_ps[:], WASB[l][g][:], hown[:],
                                     start=True, stop=True)
                    nc.scalar.copy(ST[g][:], st_ps[:])
                    nc.scalar.activation(RR[g][:], ST[g][0:HEADS, :], AF.Exp,
                                         scale=0.8)
                    if DEBUG and l == 0 and g == 0:
                        nc.gpsimd.dma_start(out=dbg["d_st"].ap(), in_=ST[0][:])
                        nc.gpsimd.dma_start(out=dbg["d_rr"].ap(), in_=RR[0][:])

                # weight builds
                for g in range(2):
                    nc.gpsimd.tensor_tensor(
                        out=GW_v[g][:, :, :, 0:16],
                        in0=WH_v[:, :, GOFF[g]:GOFF[g] + HID].rearrange(
                            "p k (h d) -> p k h d", d=HD),
                        in1=QQ_v[:, :, g, 1, :][:, :, :, None].to_broadcast(
                            (P, UC, HEADS, HD)),
                        op=op.mult)
                    nc.gpsimd.tensor_copy(out=GW_v[g][:, :, :, 16],
                                          in_=QQ_v[:, :, g, 1, :])
                    for h in range(HEADS):
                        nc.gpsimd.tensor_tensor(
                            out=WTu_v[g][h][:, :, :, 0:16],
                            in0=WH_v[:, :, GOFF[g] + HD * h:
                                     GOFF[g] + HD * h + HD][:, :, None, :]
                                .to_broadcast((P, UC, 2, HD)),
                            in1=QQ_v[:, :, g, :, h][:, :, :, None].to_broadcast(
                                (P, UC, 2, HD)),
                            op=op.mult)
                        nc.gpsimd.tensor_copy(out=WTu_v[g][h][:, :, :, 16],
                                              in_=QQ_v[:, :, g, :, h])

                if l == 0:
                    for g in range(2):
                        nc.gpsimd.dma_start(out=adjTB[g][:], in_=adj_d[g].ap())

                # attention units: c = 1{s+t>0} via 4x TS, mask via one
                # head-batched 2x TT per chunk, then 4 matmuls.
                for g in range(2):
                    sbs = []
                    for h in range(HEADS):
                        sb_ps = ps_st.tile([P, VL], dt.float32, tag="stwh")
                        nc.tensor.matmul(sb_ps[:],
                                         SEL8[:, P * h:P * (h + 1)],
                                         ST[g][:], start=True, stop=True)
                        s_b = nsp.tile([P, VL], dt.bfloat16, tag="ns_b")
                        nc.scalar.copy(s_b[:], sb_ps[:])
                        sbs.append(s_b)

                    psum_cs = []
                    for h in range(HEADS):
                        psum_c = ps_c.tile([2 * BLK, VL], dt.float32,
                                           tag="psum_c")
                        psum_cs.append(psum_c)
                    for k in range(UC):
                        veng = nc.vector
                        cb4 = chp.tile([P, HEADS, VL], dt.bfloat16, tag="cb4")
                        for h in range(HEADS):
                            nc.vector.tensor_scalar(
                                cb4[:, h, :], sbs[h][:],
                                WH_v[:, k, TOFF[g] + h:TOFF[g] + h + 1], 0.0,
                                op.add, op.is_gt)
                        chat4 = chp.tile([P, HEADS, VL], dt.bfloat16, tag="chat4")
                        veng.tensor_tensor(
                            out=chat4[:], in0=cb4[:],
                            in1=adj_v[g][:, k, :][:, None, :].to_broadcast(
                                (P, HEADS, VL)),
                            op=op.mult)
                        for h in range(HEADS):
                            nc.tensor.matmul(psum_cs[h][:],
                                             WTu_v[g][h][:, k, :, :],
                                             chat4[:, h, :], start=(k == 0),
                                             stop=(k == UC - 1))
                    for h in range(HEADS):
                        nc.scalar.copy(CM1[g][BLK * h:BLK * (h + 1), :],
                                       psum_cs[h][0:BLK, :])
                        nc.scalar.copy(CM3[g][BLK * h:BLK * (h + 1), :],
                                       psum_cs[h][BLK:2 * BLK, :])

                    # G-stream: rhs is the resident {0,1} bf16 adjacency
                    psum_g = ps_c.tile([FP, VL], dt.float32, tag="psum_c")
                    for k in range(UC):
                        nc.tensor.matmul(psum_g[:], GW_v[g][:, k, :, :],
                                         adj_v[g][:, k, :], start=(k == 0),
                                         stop=(k == UC - 1))

                    # epilogue (batched over the 4 heads)
                    nc.scalar.copy(CPG[g][:], psum_g[:])
                    if DEBUG and l == 0 and g == 0:
                        nc.sync.dma_start(out=dbg["d_cm1"].ap(), in_=CM1[0][:])
                        nc.sync.dma_start(out=dbg["d_cm3"].ap(), in_=CM3[0][:])
                        nc.sync.dma_start(out=dbg["d_cpg"].ap(), in_=CPG[0][:])
                    t4 = wp.tile([FP, VL], dt.float32, tag="w")
                    nc.vector.tensor_tensor(out=t4[:], in0=CPG[g][:],
                                            in1=CM3[g][:], op=op.subtract)
                    rb_ps = ps_m.tile([FP, VL], dt.float32, tag="m")
                    nc.tensor.matmul(rb_ps[:], E17F[:], RR[g][:],
                                     start=True, stop=True)
                    m1r = wp.tile([FP, VL], dt.float32, tag="w")
                    nc.vector.tensor_tensor(out=m1r[:], in0=CM1[g][:],
                                            in1=rb_ps[:], op=op.mult)
                    xx = wp.tile([FP, VL], dt.float32, tag="w")
                    nc.vector.tensor_tensor(out=xx[:], in0=t4[:], in1=m1r[:],
                                            op=op.add)
                    den4 = smp.tile([HEADS, VL], dt.float32, tag="s")
                    nc.sync.dma_start(out=den4[:], in_=xx[16::BLK, :])
                    rda = smp.tile([HEADS, VL], dt.float32, tag="s")
                    nc.vector.reciprocal(rda[:], den4[:])
                    rd_ps = ps_m.tile([FP, VL], dt.float32, tag="m")
                    nc.tensor.matmul(rd_ps[:], E17F[:], rda[:],
                                     start=True, stop=True)
                    hgx = wp.tile([FP, VL], dt.float32, tag="w")
                    nc.vector.tensor_tensor(out=hgx[:], in0=xx[:], in1=rd_ps[:],
                                            op=op.mult)
                    if DEBUG and l == 0 and g == 0:
                        nc.sync.dma_start(out=dbg["d_xx"].ap(), in_=xx[:])
                        nc.sync.dma_start(out=dbg["d_hgx"].ap(), in_=hgx[:])

                    # elu
                    r0 = wp.tile([FP, VL], dt.float32, tag="w")
                    nc.scalar.activation(r0[:], hgx[:], AF.Relu)
                    rn = wp.tile([FP, VL], dt.float32, tag="w")
                    nc.scalar.activation(rn[:], hgx[:], AF.Relu, scale=-1.0)
                    em = wp.tile([FP, VL], dt.float32, tag="w")
                    nc.scalar.activation(em[:], rn[:], AF.Exp, scale=-1.0)
                    nc.vector.scalar_tensor_tensor(
                        out=HE[g][:], in0=r0[:], scalar=-1.0, in1=em[:],
                        op0=op.add, op1=op.add)

                if DEBUG and l == 0:
                    nc.sync.dma_start(out=dbg["d_he0"].ap(), in_=HE[0][:])
                    nc.sync.dma_start(out=dbg["d_he1"].ap(), in_=HE[1][:])

                # fuse
                ei = []
                for g in range(2):
                    ai_ps = ps_m.tile([1, VL], dt.float32, tag="m")
                    nc.tensor.matmul(ai_ps[:], QG[l][:, g:g + 1], HE[g][:],
                                     start=True, stop=True)
                    e = smp.tile([1, VL], dt.float32, tag="s")
                    nc.scalar.activation(e[:], ai_ps[:], AF.Exp)
                    ei.append(e)
                dsum = smp.tile([1, VL], dt.float32, tag="s")
                nc.vector.tensor_tensor(out=dsum[:], in0=ei[0][:], in1=ei[1][:],
                                        op=op.add)
                rdf = smp.tile([1, VL], dt.float32, tag="s")
                nc.vector.reciprocal(rdf[:], dsum[:])
                b0 = smp.tile([1, VL], dt.float32, tag="s")
                nc.vector.tensor_tensor(out=b0[:], in0=ei[0][:], in1=rdf[:],
                                        op=op.mult)
                bib_ps = ps_m.tile([FP, VL], dt.float32, tag="m")
                nc.tensor.matmul(bib_ps[:], ONES68[:], b0[:],
                                 start=True, stop=True)
                dd = wp.tile([FP, VL], dt.float32, tag="w")
                nc.vector.tensor_tensor(out=dd[:], in0=HE[0][:], in1=HE[1][:],
                                        op=op.subtract)
                bd = wp.tile([FP, VL], dt.float32, tag="w")
                nc.vector.tensor_tensor(out=bd[:], in0=dd[:], in1=bib_ps[:],
                                        op=op.mult)
                nc.vector.tensor_tensor(out=hf_out[:], in0=HE[1][:], in1=bd[:],
                                        op=op.add)

            # ---------------- hop 1 ----------------
            prep_weights(0)
            prep_weights(1)
            layer(0, xT, XOWN, HF1)

            # all-gather H1 (feature-major)
            ag_in = drp.tile([FP, VL], dt.float32)
            ag_out = drp.tile([NCORES, FP, VL], dt.float32)
            nc.gpsimd.dma_start(out=ag_in[:], in_=HF1[:])
            if NO_COLLECTIVE:
                for c in range(NCORES):
                    nc.gpsimd.dma_start(
                        out=ag_out.opt().rearrange("c (f v) -> c f v", v=VL)[c],
                        in_=ag_in[:])
            else:
                nc.gpsimd.collective_compute(
                    "AllGather", op.bypass,
                    replica_groups=[list(range(NCORES))],
                    ins=[ag_in.opt()], outs=[ag_out.opt()])
            agv = ag_out.opt().rearrange("c (f v) -> c f v", v=VL)
            h1v = H1T.rearrange("f (c v) -> f c v", v=VL)
            for c in range(NCORES):
                nc.sync.dma_start(out=h1v[:, c, :], in_=agv[c])

            if DEBUG:
                nc.gpsimd.dma_start(out=dbg["d_hf1"].ap(), in_=HF1[:])
                nc.gpsimd.dma_start(out=dbg["d_h1t"].ap(), in_=H1T[:])

            # ---------------- hop 2 ----------------
            layer(1, H1T, HF1, HF2)

            # ---------------- MLP head ----------------
            h_ps = ps_m.tile([MH, VL], dt.float32, tag="m")
            nc.tensor.matmul(h_ps[:], MW1[:], HF2[:], start=True, stop=True)
            hd = smp.tile([MH, VL], dt.float32, tag="s")
            nc.scalar.activation(hd[:], h_ps[:], AF.Relu, bias=MB1[:])
            o_ps = ps_m.tile([1, VL], dt.float32, tag="m")
            nc.tensor.matmul(o_ps[:], MW2[:], hd[:], start=True, stop=True)
            osb = smp.tile([1, VL], dt.float32, tag="s")
            nc.scalar.activation(osb[:], o_ps[:], AF.Identity, bias=MB2[:])
            nc.sync.dma_start(out=out_d.ap(), in_=osb[:])

    nc.compile()
    return nc


def _pad_rows(w):
    out = np.zeros((FP,) + w.shape[1:], dtype=np.float32)
    for h in range(HEADS):
        out[BLK * h:BLK * h + 16] = w[16 * h:16 * h + 16]
    return out


def _ahat(a):
    A = np.zeros((HID, 2 * HEADS), dtype=np.float32)
    for h in range(HEADS):
        A[16 * h:16 * h + 16, h] = a[h, :HD]
        A[16 * h:16 * h + 16, HEADS + h] = a[h, HD:]
    return A


def _prep_adj(adj, c):
    """(N,N) int -> per-core (P, UC*VL) bf16 {0,1} chunk layout of adjT."""
    sl = adj[c * VL:(c + 1) * VL, :].T.astype(np.float32)       # (N, VL)
    sl = sl.reshape(UC, P, VL).transpose(1, 0, 2).reshape(P, UC * VL)
    return np.ascontiguousarray(sl).astype(ml_dtypes.bfloat16)


def kernel(**inputs):
    from concourse.bass_utils import run_bass_kernel_spmd

    if "nc" not in _CACHE:
        _CACHE["nc"] = _build()
    nc = _CACHE["nc"]

    f32 = np.float32
    x = np.asarray(inputs["x"], f32)
    adj = [np.asarray(inputs["adj_ind"]), np.asarray(inputs["adj_cor"])]
    W1 = [np.asarray(inputs["W1i"], f32), np.asarray(inputs["W1c"], f32)]
    W2 = [np.asarray(inputs["W2i"], f32), np.asarray(inputs["W2c"], f32)]
    A1 = [np.asarray(inputs["a1i"], f32), np.asarray(inputs["a1c"], f32)]
    A2 = [np.asarray(inputs["a2i"], f32), np.asarray(inputs["a2c"], f32)]
    q1 = [np.asarray(inputs["q1i"], f32), np.asarray(inputs["q1c"], f32)]
    q2 = [np.asarray(inputs["q2i"], f32), np.asarray(inputs["q2c"], f32)]

    common = {"xT": np.ascontiguousarray(x.T)}
    for l, (Ws, As) in enumerate(((W1, A1), (W2, A2))):
        for g in range(2):
            W = Ws[g] if l == 0 else _pad_rows(Ws[g])
            common[f"W{l}{g}"] = np.ascontiguousarray(W)
            common[f"WT{l}{g}"] = np.ascontiguousarray(W.T)
            common[f"A{l}{g}"] = _ahat(As[g])
    for l, qs in enumerate((q1, q2)):
        common[f"qg{l}"] = np.ascontiguousarray(
            np.stack([_pad_rows(qs[0][:, None])[:, 0],
                      _pad_rows(qs[1][:, None])[:, 0]], axis=1))
    common["mw1"] = _pad_rows(np.asarray(inputs["mlp_w1"], f32))
    common["mb1"] = np.ascontiguousarray(np.asarray(inputs["mlp_b1"], f32)[:, None])
    common["mw2"] = np.ascontiguousarray(np.asarray(inputs["mlp_w2"], f32))
    common["mb2"] = np.asarray(inputs["mlp_b2"], f32).reshape(1, 1)

    in_maps = []
    for c in range(NCORES):
        m = dict(common)
        m["xOwnT"] = np.ascontiguousarray(x[c * VL:(c + 1) * VL, :].T)
        m["adjTB_i"] = _prep_adj(adj[0], c)
        m["adjTB_c"] = _prep_adj(adj[1], c)
        in_maps.append(m)

    res = run_bass_kernel_spmd(nc, in_maps, core_ids=list(range(NCORES)))
    out = np.concatenate([r["out"][0] for r in res.results])[:, None]
    return out.astype(np.float32)


if __name__ == "__main__":
    _CACHE["nc"] = _build()
    print("build ok")
